# revision 1
# baseline (speedup 1.0000x reference)
"""DecoderLSTM Trainium2 kernel.

Data-parallel over batch: B=512 is sharded 64-per-core across 8 NeuronCores;
LSTM/FC weights are replicated and streamed from HBM each step (63 MB fp32
exceeds the 24 MB SBUF).  All matmuls run in fp32: the 96-step recurrence is
chaotic (measured error amplification ~250x), so reduced-precision matmuls
diverge (bf16 -> 60% rel err, fp32r -> 2.4%); fp32 lands ~1e-4.

Per-core layout:
  Big matmuls ("orientation A"): out[b, feat] accumulates in PSUM, lhsT =
  transposed activations [K, 64] stationary, rhs = streamed weight tiles
  [K, 512] moving.  Gate biases are added during PSUM evacuation on DVE.
  Small matmuls (embedding, fc2) run "orientation B" (weights stationary),
  producing transposed outputs directly — y feeds back as x with no
  transpose.  h0/h1/relu transposes use the PE transpose path.

Self-contained: shapes hardcoded; nothing read from the problem directory.
"""
from contextlib import ExitStack

import numpy as np

import concourse.bass as bass
import concourse.tile as tile
from concourse import bacc, mybir
from concourse import bass_utils

F32 = mybir.dt.float32
AF = mybir.ActivationFunctionType
ALU = mybir.AluOpType

B, D, E, H, T_FULL = 512, 64, 512, 1024, 96
NC = 8
BC = B // NC          # 64 batch rows per core
KC_E = E // 128       # 4
KC_H = H // 128       # 8
G4H = 4 * H           # 4096
LN_EPS = 1e-5

_cache = {}


def _emit(ctx: ExitStack, tc: tile.TileContext, io: dict, t_steps: int):
    nc = tc.nc

    res = ctx.enter_context(tc.tile_pool(name="resident", bufs=1))
    state = ctx.enter_context(tc.tile_pool(name="state", bufs=2))
    work = ctx.enter_context(tc.tile_pool(name="work", bufs=1))
    wstream = ctx.enter_context(tc.tile_pool(name="wstream", bufs=3))
    gpsum = ctx.enter_context(tc.tile_pool(name="gpsum", bufs=4, space="PSUM"))
    tpsum = ctx.enter_context(tc.tile_pool(name="tpsum", bufs=2, space="PSUM"))
    spsum = ctx.enter_context(tc.tile_pool(name="spsum", bufs=2, space="PSUM"))

    # ---- resident tensors (loaded once) ----
    emb_W = res.tile([64, E], F32)               # [D, E]; lhsT chunks [:, c*128:]
    fc2_W = res.tile([128, KC_H, 64], F32)       # fc2 lhsT chunks
    ident = res.tile([64, 64], F32)
    b0_bc = res.tile([BC, G4H], F32)             # gate biases bcast over batch
    b1_bc = res.tile([BC, G4H], F32)
    fc1_b_bc = res.tile([BC, H], F32)
    emb_bT = res.tile([128, KC_E], F32)          # per-partition bias, chunk c
    fc2_bT = res.tile([64, 1], F32)
    g_bc = res.tile([BC, H], F32)
    bb_bc = res.tile([BC, H], F32)

    for name, t in [("emb_W", emb_W), ("ident", ident), ("b0_bc", b0_bc),
                    ("b1_bc", b1_bc), ("fc1_b_bc", fc1_b_bc),
                    ("emb_bT", emb_bT), ("fc2_bT", fc2_bT), ("g_bc", g_bc),
                    ("bb_bc", bb_bc)]:
        nc.sync.dma_start(t[:], io[name].ap())
    nc.sync.dma_start(fc2_W[:], io["fc2_W"].ap().rearrange("(k p) o -> p k o", p=128))

    # ---- initial state ----
    xT0 = res.tile([64, BC], F32)
    nc.sync.dma_start(xT0[:], io["xT0"].ap())
    eps_t = res.tile([BC, 1], F32)
    nc.vector.memset(eps_t[:], LN_EPS)
    h0T = state.tile([128, KC_H, BC], F32, tag="h0T")
    h1T = state.tile([128, KC_H, BC], F32, tag="h1T")
    c0 = state.tile([BC, H], F32, tag="c0")
    c1 = state.tile([BC, H], F32, tag="c1")
    nc.sync.dma_start(h0T[:], io["h0T0"].ap().rearrange("(k p) b -> p k b", p=128))
    nc.sync.dma_start(h1T[:], io["h1T0"].ap().rearrange("(k p) b -> p k b", p=128))
    nc.sync.dma_start(c0[:], io["c00"].ap())
    nc.sync.dma_start(c1[:], io["c10"].ap())

    y_last = None

    def lstm_layer(layer, xe_lhsT, hT_prev, c_prev, w_in_dram, w_hh_dram,
                   b_bc, kc_in):
        """Gates + cell update.  Returns (h_new [BC,H] sbuf, c_new)."""
        gts = work.tile([BC, G4H], F32, tag=f"gts{layer}")
        for half in range(2):
            gb = [gpsum.tile([BC, 512], F32, tag="gb", name=f"gb{half}_{_n}")
                  for _n in range(4)]
            # recurrent part first (hT_prev ready since last step)
            for k in range(KC_H):
                wt = wstream.tile([128, 2048], F32, tag="wstream")
                nc.sync.dma_start(
                    wt[:], w_hh_dram.ap()[k * 128:(k + 1) * 128,
                                          half * 2048:(half + 1) * 2048])
                for n in range(4):
                    nc.tensor.matmul(gb[n][:], hT_prev[:, k, :],
                                     wt[:, n * 512:(n + 1) * 512],
                                     start=(k == 0), stop=False)
            # input part
            for k in range(kc_in):
                wt = wstream.tile([128, 2048], F32, tag="wstream")
                nc.sync.dma_start(
                    wt[:], w_in_dram.ap()[k * 128:(k + 1) * 128,
                                          half * 2048:(half + 1) * 2048])
                lhsT = xe_lhsT(k)
                for n in range(4):
                    nc.tensor.matmul(gb[n][:], lhsT,
                                     wt[:, n * 512:(n + 1) * 512],
                                     start=False, stop=(k == kc_in - 1))
            # evacuate with bias add (DVE), then in-place nonlinearity (ACT)
            for n in range(4):
                col = half * 2048 + n * 512
                nc.vector.tensor_add(gts[:, col:col + 512], gb[n][:],
                                     b_bc[:, col:col + 512])
        # i f g o, each H wide
        nc.scalar.activation(gts[:, 0:2 * H], gts[:, 0:2 * H], AF.Sigmoid)
        nc.scalar.activation(gts[:, 2 * H:3 * H], gts[:, 2 * H:3 * H], AF.Tanh)
        nc.scalar.activation(gts[:, 3 * H:], gts[:, 3 * H:], AF.Sigmoid)

        c_new = state.tile([BC, H], F32, tag=f"c{layer}")
        tmp1 = work.tile([BC, H], F32, tag="tmp1")
        tanh_c = work.tile([BC, H], F32, tag=f"tanh_c{layer}")
        h_new = work.tile([BC, H], F32, tag=f"h{layer}")
        nc.vector.tensor_mul(tmp1[:], gts[:, H:2 * H], c_prev[:])
        nc.vector.tensor_mul(c_new[:], gts[:, 0:H], gts[:, 2 * H:3 * H])
        nc.vector.tensor_add(c_new[:], c_new[:], tmp1[:])
        nc.scalar.activation(tanh_c[:], c_new[:], AF.Tanh)
        nc.vector.tensor_mul(h_new[:], gts[:, 3 * H:], tanh_c[:])
        return h_new, c_new

    def transpose_to(hT_new, h_sb):
        """h [BC, H] -> hT [128, KC_H, BC] via PE transposes."""
        for ck in range(KC_H):
            tp = tpsum.tile([128, BC], F32, tag="tp")
            nc.tensor.transpose(tp[:], h_sb[:, ck * 128:(ck + 1) * 128],
                                ident[:])
            nc.vector.tensor_copy(hT_new[:, ck, :], tp[:])

    for t in range(t_steps):
        xT = xT0[:] if t == 0 else y_last[:]

        # ---- embedding (orientation B): xeT[c] = emb_W[:,c].T @ xT ----
        xeT = work.tile([128, KC_E, BC], F32, tag="xeT")
        for c in range(KC_E):
            xp = spsum.tile([128, BC], F32, tag="sp")
            nc.tensor.matmul(xp[:], emb_W[:, c * 128:(c + 1) * 128], xT,
                             start=True, stop=True)
            nc.vector.tensor_scalar_add(xeT[:, c, :], xp[:], emb_bT[:, c:c + 1])

        # ---- LSTM layers ----
        h0_new, c0_new = lstm_layer(
            0, lambda k: xeT[:, k, :], h0T, c0,
            io["W_ih0"], io["W_hh0"], b0_bc, KC_E)
        h0T_new = state.tile([128, KC_H, BC], F32, tag="h0T")
        transpose_to(h0T_new, h0_new)

        h1_new, c1_new = lstm_layer(
            1, lambda k: h0T_new[:, k, :], h1T, c1,
            io["W_ih1"], io["W_hh1"], b1_bc, KC_H)
        h1T_new = state.tile([128, KC_H, BC], F32, tag="h1T")
        transpose_to(h1T_new, h1_new)

        # ---- fc1 + LayerNorm + ReLU ----
        z = work.tile([BC, H], F32, tag="z")
        z_sums = work.tile([BC, 2], F32, tag="z_sums")
        zp = [spsum.tile([BC, 512], F32, tag="sp", name=f"zp{_n}")
               for _n in range(2)]
        for k in range(KC_H):
            wt = wstream.tile([128, H], F32, tag="wstream")
            nc.sync.dma_start(wt[:],
                              io["fc1_W"].ap()[k * 128:(k + 1) * 128, :])
            for n in range(2):
                nc.tensor.matmul(zp[n][:], h1T_new[:, k, :],
                                 wt[:, n * 512:(n + 1) * 512],
                                 start=(k == 0), stop=(k == KC_H - 1))
        for n in range(2):
            nc.vector.tensor_add(z[:, n * 512:(n + 1) * 512], zp[n][:],
                                 fc1_b_bc[:, n * 512:(n + 1) * 512])
            nc.vector.reduce_sum(z_sums[:, n:n + 1],
                                 z[:, n * 512:(n + 1) * 512],
                                 axis=mybir.AxisListType.X)
        mu = work.tile([BC, 1], F32, tag="mu")
        negmu = work.tile([BC, 1], F32, tag="negmu")
        sqs = work.tile([BC, 1], F32, tag="sqs")
        sq = work.tile([BC, H], F32, tag="sq")
        va = work.tile([BC, 1], F32, tag="va")
        sv = work.tile([BC, 1], F32, tag="sv")
        rstd = work.tile([BC, 1], F32, tag="rstd")
        nc.vector.tensor_add(mu[:], z_sums[:, 0:1], z_sums[:, 1:2])
        nc.vector.tensor_scalar_mul(negmu[:], mu[:], -1.0 / H)
        nc.vector.tensor_scalar_mul(mu[:], mu[:], 1.0 / H)
        nc.scalar.activation(sq[:], z[:], AF.Square, bias=negmu[:],
                             accum_out=sqs[:])
        nc.vector.tensor_scalar_mul(va[:], sqs[:], 1.0 / H)
        nc.scalar.activation(sv[:], va[:], AF.Sqrt, bias=eps_t[:])
        nc.vector.reciprocal(rstd[:], sv[:])
        zn = work.tile([BC, H], F32, tag="zn")
        nc.vector.tensor_scalar(zn[:], z[:], mu[:], rstd[:],
                                ALU.subtract, ALU.mult)
        nc.vector.tensor_mul(zn[:], zn[:], g_bc[:])
        nc.vector.tensor_add(zn[:], zn[:], bb_bc[:])
        nc.scalar.activation(zn[:], zn[:], AF.Relu)

        reluT = work.tile([128, KC_H, BC], F32, tag="reluT")
        transpose_to(reluT, zn)

        # ---- fc2 (orientation B): yT = fc2_W.T @ reluT ----
        yp = spsum.tile([64, BC], F32, tag="sp")
        for k in range(KC_H):
            nc.tensor.matmul(yp[:], fc2_W[:, k, :], reluT[:, k, :],
                             start=(k == 0), stop=(k == KC_H - 1))
        y_new = state.tile([64, BC], F32, tag="ylast")
        nc.vector.tensor_scalar_add(y_new[:], yp[:], fc2_bT[:])
        nc.sync.dma_start(io["ysT"].ap()[:, t, :], y_new[:])

        h0T, h1T, c0, c1, y_last = h0T_new, h1T_new, c0_new, c1_new, y_new


def build(t_steps=T_FULL):
    if t_steps in _cache:
        return _cache[t_steps]
    nc = bacc.Bacc("TRN2", target_bir_lowering=False, debug=False)
    io = {}
    inputs = [
        ("xT0", (64, BC)), ("h0T0", (H, BC)), ("h1T0", (H, BC)),
        ("c00", (BC, H)), ("c10", (BC, H)),
        ("W_ih0", (E, G4H)), ("W_hh0", (H, G4H)),
        ("W_ih1", (H, G4H)), ("W_hh1", (H, G4H)),
        ("fc1_W", (H, H)), ("fc2_W", (H, 64)), ("emb_W", (64, E)),
        ("b0_bc", (BC, G4H)), ("b1_bc", (BC, G4H)),
        ("fc1_b_bc", (BC, H)), ("emb_bT", (128, KC_E)),
        ("fc2_bT", (64, 1)), ("g_bc", (BC, H)), ("bb_bc", (BC, H)),
        ("ident", (64, 64)),
    ]
    for name, shape in inputs:
        io[name] = nc.dram_tensor(name, shape, F32, kind="ExternalInput")
    io["ysT"] = nc.dram_tensor("ysT", (64, t_steps, BC), F32,
                               kind="ExternalOutput")
    with tile.TileContext(nc) as tc:
        with ExitStack() as ctx:
            _emit(ctx, tc, io, t_steps)
    nc.compile()
    _cache[t_steps] = (nc, io)
    return nc, io


def make_in_maps(inputs):
    """Shard + transform full inputs into 8 per-core input maps."""
    f = lambda x: np.ascontiguousarray(np.asarray(x), dtype=np.float32)
    x0 = f(inputs["x_0"])
    hn = f(inputs["h_n"])
    cn = f(inputs["c_n"])
    base = {
        "W_ih0": f(inputs["W_ih0"]), "W_hh0": f(inputs["W_hh0"]),
        "W_ih1": f(inputs["W_ih1"]), "W_hh1": f(inputs["W_hh1"]),
        "fc1_W": f(inputs["fc1_W"]), "fc2_W": f(inputs["fc2_W"]),
        "emb_W": f(inputs["emb_W"]),
        "b0_bc": np.tile((f(inputs["b_ih0"]) + f(inputs["b_hh0"]))[None, :],
                         (BC, 1)),
        "b1_bc": np.tile((f(inputs["b_ih1"]) + f(inputs["b_hh1"]))[None, :],
                         (BC, 1)),
        "fc1_b_bc": np.tile(f(inputs["fc1_b"])[None, :], (BC, 1)),
        "emb_bT": np.ascontiguousarray(f(inputs["emb_b"]).reshape(KC_E, 128).T),
        "fc2_bT": f(inputs["fc2_b"])[:, None],
        "g_bc": np.tile(f(inputs["ln_g"])[None, :], (BC, 1)),
        "bb_bc": np.tile(f(inputs["ln_b"])[None, :], (BC, 1)),
        "ident": np.eye(64, dtype=np.float32),
    }
    in_maps = []
    for c in range(NC):
        sl = slice(c * BC, (c + 1) * BC)
        m = dict(base)
        m["xT0"] = np.ascontiguousarray(x0[sl].T)
        m["h0T0"] = np.ascontiguousarray(hn[0, sl].T)
        m["h1T0"] = np.ascontiguousarray(hn[1, sl].T)
        m["c00"] = np.ascontiguousarray(cn[0, sl])
        m["c10"] = np.ascontiguousarray(cn[1, sl])
        in_maps.append(m)
    return in_maps


def kernel(**inputs):
    t_steps = int(inputs.get("forecast_window", T_FULL))
    nc, io = build(t_steps)
    in_maps = make_in_maps(inputs)
    r = bass_utils.run_bass_kernel_spmd(nc, in_maps, core_ids=list(range(NC)))
    out = np.empty((B, t_steps, D), np.float32)
    for c in range(NC):
        ysT = r.results[c]["ysT"]              # [D, t, BC]
        out[c * BC:(c + 1) * BC] = ysT.transpose(2, 1, 0)
    return out

